# revision 2
# baseline (speedup 1.0000x reference)
"""Bass/Trainium2 kernel for nn_AvgPoolBackbone (segment_reduce).

Computes, for each batch row b of x [B, S, D]:
    eff = S if idx[b] == -1 else idx[b]
    out[b] = mean(x[b, :eff], axis=0)   (zeros when eff <= 0)

Strategy
--------
Pure data parallel over 8 NeuronCores (16 batches each).  On the host we
fold the prefix mask AND the 1/eff_len scaling into a single f32 matrix
`maskt` laid out as [128 partitions, (batch, s_tile)] so the device does
no division and no control flow.  On the device each (batch, s-tile)
contributes one tiny TensorE matmul:

    psum[1, D] += maskt[:, col].T @ x_tile[128 seq, D]

accumulated over the 16 s-tiles of a batch in PSUM.  TensorE performs the
masking and the cross-partition reduction in one instruction, so the
kernel is purely HBM-bandwidth bound (32 MiB of x per core).
"""

import numpy as np

import concourse.bass as bass
import concourse.tile as tile
from concourse import bacc, mybir
from concourse import bass_utils

F32 = mybir.dt.float32

# Problem config (hardcoded per the harness contract).
B, S, D = 128, 2048, 256
N_CORES = 8
BL = B // N_CORES  # batches per core
P = 128            # SBUF partitions / seq-tile height
T = S // P         # s-tiles per batch
CHUNK_B = 2        # batches loaded per DMA (CHUNK_B*S*D*4 bytes per transfer)


def build_kernel(bl=BL, s=S, d=D, chunk_b=CHUNK_B):
    """Build + compile the single-core Bass module (same NEFF on all cores)."""
    t = s // P
    nc = bacc.Bacc("TRN2", target_bir_lowering=False, debug=False)
    x = nc.dram_tensor("x", (bl, s, d), F32, kind="ExternalInput")
    maskt = nc.dram_tensor("maskt", (P, bl * t), F32, kind="ExternalInput")
    out = nc.dram_tensor("out", (1, bl * d), F32, kind="ExternalOutput")

    with tile.TileContext(nc) as tc:
        with (
            tc.tile_pool(name="xp", bufs=3) as xp,
            tc.tile_pool(name="mp", bufs=1) as mp,
            tc.tile_pool(name="op", bufs=1) as op,
            tc.tile_pool(name="ps", bufs=8, space=bass.MemorySpace.PSUM) as ps,
        ):
            m_t = mp.tile([P, bl * t], F32)
            nc.sync.dma_start(m_t[:], maskt.ap())
            o_t = op.tile([1, bl * d], F32)
            for b0 in range(0, bl, chunk_b):
                nb = min(chunk_b, bl - b0)
                x_t = xp.tile([P, nb, t, d], F32)
                # sbuf[p, b, t, d] = x[b0+b, t*128+p, d]
                nc.sync.dma_start(
                    x_t[:],
                    x.ap()[b0 : b0 + nb].rearrange("b (t p) d -> p b t d", p=P),
                )
                for bi in range(nb):
                    b = b0 + bi
                    acc = ps.tile([1, d], F32)
                    for ti in range(t):
                        col = b * t + ti
                        nc.tensor.matmul(
                            acc[:],
                            m_t[:, col : col + 1],
                            x_t[:, bi, ti, :],
                            start=(ti == 0),
                            stop=(ti == t - 1),
                        )
                    nc.vector.tensor_copy(o_t[:, b * d : (b + 1) * d], acc[:])
            nc.sync.dma_start(out.ap(), o_t[:])

    nc.compile()
    return nc


def make_host_inputs(x, start_padding_indices, n_cores=N_CORES, bl=BL, s=S, d=D):
    """Shard x and build the per-core scaled+transposed mask matrices."""
    x = np.ascontiguousarray(np.asarray(x, dtype=np.float32))
    idx = np.asarray(start_padding_indices).astype(np.int64)
    t = s // P
    eff = np.where(idx == -1, s, idx).astype(np.int64)  # [B]
    scale = 1.0 / np.maximum(eff, 1).astype(np.float64)
    mask = (np.arange(s)[None, :] < eff[:, None]) * scale[:, None]  # [B, S] f64
    mask = mask.astype(np.float32)
    # [B, S] -> per batch [t, P] -> transpose to [P, t]; pack cores
    mask_pt = mask.reshape(-1, t, P).transpose(0, 2, 1)  # [B, P, t]
    in_maps = []
    for c in range(n_cores):
        mb = mask_pt[c * bl : (c + 1) * bl]  # [bl, P, t]
        maskt = np.ascontiguousarray(mb.transpose(1, 0, 2).reshape(P, bl * t))
        in_maps.append(
            {
                "x": np.ascontiguousarray(x[c * bl : (c + 1) * bl]),
                "maskt": maskt,
            }
        )
    return in_maps


_CACHED_NC = None


def _get_nc():
    global _CACHED_NC
    if _CACHED_NC is None:
        _CACHED_NC = build_kernel()
    return _CACHED_NC


def run(x, start_padding_indices, trace=False):
    """Run on all 8 cores; returns (out [B, D] f32, BassKernelResults)."""
    nc = _get_nc()
    in_maps = make_host_inputs(x, start_padding_indices)
    res = bass_utils.run_bass_kernel_spmd(
        nc, in_maps, core_ids=list(range(N_CORES)), trace=trace
    )
    outs = [r["out"].reshape(BL, D) for r in res.results]
    return np.concatenate(outs, axis=0), res


def kernel(x, start_padding_indices):
    out, _ = run(x, start_padding_indices, trace=False)
    return out


# revision 4
# speedup vs baseline: 1.3635x; 1.3635x over previous
"""Bass/Trainium2 kernel for nn_AvgPoolBackbone (segment_reduce).

Computes, for each batch row b of x [B, S, D]:
    eff = S if idx[b] == -1 else idx[b]
    out[b] = mean(x[b, :eff], axis=0)   (zeros when eff <= 0)

Strategy
--------
Pure data parallel over 8 NeuronCores (16 batches each).  On the host we
fold the prefix mask AND the 1/eff_len scaling into a single f32 matrix
`maskt` so the device does no division and no control flow.

Per batch, x[b] ([2048, 256] f32, 2 MiB) is viewed as [128, 16*256]:
partition p holds the 16 consecutive sequence rows p*16..p*16+15 — one
contiguous 16 KiB DRAM run per partition, which keeps the DMA descriptors
large.  The masked mean is then 16 PSUM-accumulated TensorE matmuls

    psum[1, D] += maskt[:, col].T @ x_view[:, j*D:(j+1)*D]

where maskt[p, col] = mask[b, p*16 + j] / eff_len[b].  Operands are
bitcast to float32r, which streams one PSUM row per cycle (4x faster
than the two-pass fp32 path) at N=256.  TensorE does the masking and the
cross-partition reduction in one instruction; the kernel is
HBM-bandwidth bound.
"""

import numpy as np

import concourse.bass as bass
import concourse.tile as tile
from concourse import bacc, mybir
from concourse import bass_utils

F32 = mybir.dt.float32
F32R = mybir.dt.float32r

# Problem config (hardcoded per the harness contract).
B, S, D = 128, 2048, 256
N_CORES = 8
BL = B // N_CORES  # batches per core
P = 128            # SBUF partitions
CHUNK_B = 2        # batches loaded per DMA


def build_kernel(bl=BL, s=S, d=D, chunk_b=CHUNK_B, f32r=True):
    """Build + compile the single-core Bass module (same NEFF on all cores)."""
    j = s // P  # seq rows per partition (16 at full size)
    mmdt = F32R if f32r else F32
    nc = bacc.Bacc("TRN2", target_bir_lowering=False, debug=False)
    x = nc.dram_tensor("x", (bl, s, d), mmdt, kind="ExternalInput")
    maskt = nc.dram_tensor("maskt", (P, bl * j), mmdt, kind="ExternalInput")
    out = nc.dram_tensor("out", (1, bl * d), F32, kind="ExternalOutput")

    with tile.TileContext(nc) as tc:
        with (
            tc.tile_pool(name="xp", bufs=3) as xp,
            tc.tile_pool(name="mp", bufs=1) as mp,
            tc.tile_pool(name="op", bufs=1) as op,
            tc.tile_pool(name="ps", bufs=8, space=bass.MemorySpace.PSUM) as ps,
        ):
            m_t = mp.tile([P, bl * j], mmdt)
            nc.sync.dma_start(m_t[:], maskt.ap())
            o_t = op.tile([1, bl * d], F32)
            for b0 in range(0, bl, chunk_b):
                nb = min(chunk_b, bl - b0)
                x_t = xp.tile([P, nb, j * d], mmdt)
                # sbuf[p, b, ji*d + di] = x[b0+b, p*j + ji, di]
                # -> per (p, b) one contiguous j*d*4-byte DRAM run
                nc.sync.dma_start(
                    x_t[:],
                    x.ap()[b0 : b0 + nb].rearrange("b (p j) d -> p b (j d)", p=P),
                )
                for bi in range(nb):
                    b = b0 + bi
                    acc = ps.tile([1, d], F32)
                    for ji in range(j):
                        col = b * j + ji
                        nc.tensor.matmul(
                            acc[:],
                            m_t[:, col : col + 1],
                            x_t[:, bi, ji * d : (ji + 1) * d],
                            start=(ji == 0),
                            stop=(ji == j - 1),
                        )
                    nc.vector.tensor_copy(o_t[:, b * d : (b + 1) * d], acc[:])
            nc.sync.dma_start(out.ap(), o_t[:])

    nc.compile()
    return nc


def make_host_inputs(x, start_padding_indices, n_cores=N_CORES, bl=BL, s=S, d=D):
    """Shard x and build the per-core scaled mask matrices.

    maskt[p, b*j + ji] = (p*j + ji < eff[b]) / max(eff[b], 1)
    """
    x = np.ascontiguousarray(np.asarray(x, dtype=np.float32))
    idx = np.asarray(start_padding_indices).astype(np.int64)
    j = s // P
    eff = np.where(idx == -1, s, idx).astype(np.int64)  # [B]
    scale = 1.0 / np.maximum(eff, 1).astype(np.float64)
    mask = (np.arange(s)[None, :] < eff[:, None]) * scale[:, None]  # [B, S] f64
    mask = mask.astype(np.float32)
    # [B, S] -> [B, P, j] (s-major within partition) -> cores pack [P, bl*j]
    mask_pj = mask.reshape(-1, P, j)  # [B, P, j]
    in_maps = []
    for c in range(n_cores):
        mb = mask_pj[c * bl : (c + 1) * bl]  # [bl, P, j]
        maskt = np.ascontiguousarray(mb.transpose(1, 0, 2).reshape(P, bl * j))
        in_maps.append(
            {
                "x": np.ascontiguousarray(x[c * bl : (c + 1) * bl]),
                "maskt": maskt,
            }
        )
    return in_maps


_CACHED_NC = None


def _get_nc():
    global _CACHED_NC
    if _CACHED_NC is None:
        _CACHED_NC = build_kernel()
    return _CACHED_NC


def run(x, start_padding_indices, trace=False):
    """Run on all 8 cores; returns (out [B, D] f32, BassKernelResults)."""
    nc = _get_nc()
    in_maps = make_host_inputs(x, start_padding_indices)
    res = bass_utils.run_bass_kernel_spmd(
        nc, in_maps, core_ids=list(range(N_CORES)), trace=trace
    )
    outs = [r["out"].reshape(BL, D) for r in res.results]
    return np.concatenate(outs, axis=0), res


def kernel(x, start_padding_indices):
    out, _ = run(x, start_padding_indices, trace=False)
    return out
